# revision 15
# baseline (speedup 1.0000x reference)
"""Causal self-attention (B=2, T=2048, C=768, H=12) on 8 TRN2 NeuronCores.

Sharding: core c -> batch b = c//4, head-group g = c%4 (heads 3g..3g+2).
Each core computes QKV for its 3 heads, causal attention, and a partial
c_proj (its heads' rows of W_proj). Host sums the 4 partials per batch.

Layout: feature-on-partitions for Q/K (S^T tiles [k, q]), but V is
computed directly in [t, d] layout (lhsT = x^T chunks), which feeds the
PV matmul with no PE transposes. Softmax over k uses an appended
ones-column on V so the PV accumulation yields [y^T; denom] in one
group. No max-subtraction: scores ~N(0,1), exp is fp32-safe.

qk m-tiles (host packs): m0=[Q0|Q1] m1=[K0|K1] m2=[Q2|K2], 128 rows
each so Q_h/K_h of heads 0/1 sit at partition bases 0/64. S matmuls for
h0 (rows 0:64) and h1 (rows 64:128) are emitted back-to-back and run
CONCURRENTLY in different PE row-groups (tile_position auto-derived),
halving S wall time. h2's K2 is copied to a base-0 scratch by GpSimd.

Engine budget per core (est): ACT ~69us (exp, the bottleneck), PE ~71us,
DVE ~50us, GpSimd ~35us. Emission software-pipelines one kt-step ahead
(S(kt+1) before PV(kt)) and a credit-based filler scheduler injects
QKV / V / proj matmul groups between attention steps so PE stays dense
while ACT runs exp. Reciprocal uses reciprocal_approx_fast (5x faster
than DVE reciprocal) on the [1,512] denom row, broadcast by GpSimd.
"""

import os

import numpy as np
import ml_dtypes

import concourse.bass as bass
import concourse.mybir as mybir
import concourse.tile as tile
from concourse import bacc
from concourse.bass_utils import run_bass_kernel_spmd
from concourse.masks import make_upper_triangular

F32 = mybir.dt.float32
F32R = mybir.dt.float32r
BF16 = mybir.dt.bfloat16
AF = mybir.ActivationFunctionType

T = 2048           # sequence length
C = 768            # embed dim
HPC = 3            # heads per core
D = 64             # head dim
QC = 512           # q-chunk (psum bank width in fp32)
KT = 128           # k-tile
NKT = T // KT      # 16
NQC = T // QC      # 4
NCH = C // 128     # 6 contraction chunks
SCALE = 1.0 / 8.0  # 1/sqrt(64)

_CACHE = {}
LAST_RESULTS = None
_TCNT = [0]
# debug bisect switches: 'r' safe reciprocal, 'm' masks on vector,
# 'b' skip V-bias matmul
KSAFE = os.environ.get("KSAFE", "")


def mk_persist(pool, shape, dtype, name=None):
    if name is None:
        _TCNT[0] += 1
        name = f"pt{_TCNT[0]}"
    return pool.tile(shape, dtype, name=name, tag=name)


def build():
    nc = bacc.Bacc("TRN2", target_bir_lowering=False)

    xTb = nc.dram_tensor("xTb", [C, T], BF16, kind="ExternalInput")
    wqk = nc.dram_tensor("wqk", [C, 384], BF16, kind="ExternalInput")
    wv = nc.dram_tensor("wv", [C, 192], BF16, kind="ExternalInput")
    battn = nc.dram_tensor("battn", [128, 3], F32, kind="ExternalInput")
    bv = nc.dram_tensor("bv", [1, 192], BF16, kind="ExternalInput")
    wp0 = nc.dram_tensor("wp0", [128, C], BF16, kind="ExternalInput")
    wp1 = nc.dram_tensor("wp1", [64, C], BF16, kind="ExternalInput")
    yT = nc.dram_tensor("yT", [C, T], F32, kind="ExternalOutput")

    with tile.TileContext(nc) as tc, \
            tc.tile_pool(name="persist", bufs=1) as pp:
        # ---- persistent SBUF tensors ----
        xsbb = mk_persist(pp, [128, NCH, T], BF16)     # x^T bf16
        wqk_sb = mk_persist(pp, [128, NCH, 384], BF16)
        wv_sb = mk_persist(pp, [128, NCH, 192], BF16)
        battn_sb = mk_persist(pp, [128, 3], F32)
        bv_sb = mk_persist(pp, [1, 192], BF16)
        ones_r = mk_persist(pp, [1, 128], BF16)        # bias-matmul lhsT
        qA = mk_persist(pp, [128, T], F32R)    # [Q0|Q1]
        kA = mk_persist(pp, [128, T], F32R)    # [K0|K1]
        mCs = mk_persist(pp, [128, T], F32R)   # [Q2|K2]
        k2s = mk_persist(pp, [64, T], F32R)    # K2 copied to base 0
        vaug = mk_persist(pp, [128, NKT, 3 * 65], BF16)  # V[t,d]+ones per kt
        yA = mk_persist(pp, [128, T], BF16)    # y^T heads 0,1
        yB = mk_persist(pp, [64, T], BF16)     # y^T head 2
        wp0_sb = mk_persist(pp, [128, C], BF16)
        wp1_sb = mk_persist(pp, [64, C], BF16)
        trimask_s = mk_persist(pp, [128, 128], F32)   # [k,q]=1 iff k<=q
        trimask = mk_persist(pp, [128, 128], BF16)
        dummy = mk_persist(pp, [1, 8], F32)

        make_upper_triangular(nc, trimask_s[:, :], val=1.0, diag=True)
        nc.vector.tensor_copy(trimask[:, :], trimask_s[:, :])
        nc.vector.memset(ones_r[:, :], 1.0)
        # preload the exp table set while DMAs run
        nc.vector.memset(dummy[:, :], 0.0)
        nc.scalar.activation(dummy[:, :], dummy[:, :], AF.Exp)
        for i in range(3):  # ones columns of vaug
            nc.vector.memset(vaug[:, :, 65 * i + 64:65 * i + 65], 1.0)

        # ---- DMA loads, earliest-needed first ----
        for cc in range(NCH):
            nc.sync.dma_start(wqk_sb[:, cc, 0:256], wqk[cc * 128:(cc + 1) * 128, 0:256])
        for cc in range(NCH):
            nc.sync.dma_start(xsbb[:, cc, 0:QC], xTb[cc * 128:(cc + 1) * 128, 0:QC])
        nc.sync.dma_start(battn_sb[:, :], battn[:, :])
        nc.sync.dma_start(bv_sb[:, :], bv[:, :])
        for cc in range(NCH):
            nc.sync.dma_start(wv_sb[:, cc, :], wv[cc * 128:(cc + 1) * 128, :])
            nc.sync.dma_start(wqk_sb[:, cc, 256:384], wqk[cc * 128:(cc + 1) * 128, 256:384])
        for t in range(1, NQC):
            for cc in range(NCH):
                nc.sync.dma_start(
                    xsbb[:, cc, t * QC:(t + 1) * QC],
                    xTb[cc * 128:(cc + 1) * 128, t * QC:(t + 1) * QC])
        nc.sync.dma_start(wp0_sb[:, :], wp0[:, :])
        nc.sync.dma_start(wp1_sb[:, :], wp1[:, :])

        qk_dest = [qA, kA, mCs]

        with (
            tc.tile_pool(name="psS", bufs=2, space="PSUM") as psS,
            tc.tile_pool(name="psY", bufs=2, space="PSUM") as psY,
            tc.tile_pool(name="psM", bufs=2, space="PSUM") as psM,
            tc.tile_pool(name="sb", bufs=8) as sbp,
        ):
            # --- emit helpers; each returns nothing, updates credit ---
            cred = {"pe": 0.0, "act": 0.0}

            def emit_qk(m, t):
                ps = psM.tile([128, QC], F32, tag="pm", name="pm")
                for cc in range(NCH):
                    nc.tensor.matmul(
                        ps[:, :],
                        lhsT=wqk_sb[:, cc, m * 128:(m + 1) * 128],
                        rhs=xsbb[:, cc, t * QC:(t + 1) * QC],
                        start=(cc == 0), stop=(cc == NCH - 1),
                    )
                nc.vector.tensor_scalar_add(
                    qk_dest[m][:, t * QC:(t + 1) * QC], ps[:, :],
                    battn_sb[:, m:m + 1],
                )
                cred["pe"] += 1350

            def emit_k2s(t):
                # partition-shifting copy (rows 64:128 -> 0:64): only DMA
                # can move data across partitions cheaply
                nc.sync.dma_start(
                    k2s[0:64, t * QC:(t + 1) * QC],
                    mCs[64:128, t * QC:(t + 1) * QC],
                )

            def emit_v(kt):
                ps = psM.tile([128, QC], F32, tag="pm", name="pm")
                skip_bias = "b" in KSAFE
                for cc in range(NCH):
                    nc.tensor.matmul(
                        ps[:, 0:192],
                        lhsT=xsbb[:, cc, kt * KT:(kt + 1) * KT],
                        rhs=wv_sb[:, cc, :],
                        start=(cc == 0), stop=(skip_bias and cc == NCH - 1),
                    )
                if not skip_bias:
                    nc.tensor.matmul(
                        ps[:, 0:192],
                        lhsT=ones_r[:, :],
                        rhs=bv_sb[:, :],
                        start=False, stop=True,
                    )
                nc.vector.tensor_copy(
                    vaug[:, kt, :].rearrange("p (a b) -> p a b", b=65)[:, :, 0:64],
                    ps[:, 0:192].rearrange("p (a b) -> p a b", b=64),
                )
                cred["pe"] += 640

            def emit_proj(ct, t, on_act=False):
                ps = psM.tile([128, QC], F32, tag="pm", name="pm")
                nc.tensor.matmul(
                    ps[:, :],
                    lhsT=wp0_sb[:, ct * 128:(ct + 1) * 128],
                    rhs=yA[:, t * QC:(t + 1) * QC],
                    start=True, stop=False,
                )
                nc.tensor.matmul(
                    ps[:, :],
                    lhsT=wp1_sb[0:64, ct * 128:(ct + 1) * 128],
                    rhs=yB[0:64, t * QC:(t + 1) * QC],
                    start=False, stop=True,
                )
                osb = sbp.tile([128, QC], F32, tag="osb", name="osb")
                if on_act:
                    nc.scalar.activation(osb[:, :], ps[:, :], AF.Copy)
                else:
                    nc.vector.tensor_copy(osb[:, :], ps[:, :])
                nc.sync.dma_start(
                    yT[ct * 128:(ct + 1) * 128, t * QC:(t + 1) * QC],
                    osb[:, :],
                )
                cred["pe"] += 500

            # ---- filler scheduler ----
            emitted = set()

            def emit_group(g):
                if g in emitted:
                    return
                emitted.add(g)
                kind = g[0]
                if kind == "qk":
                    emit_qk(g[1], g[2])
                elif kind == "v":
                    emit_v(g[1])
                elif kind == "k2s":
                    emit_k2s(g[1])
                elif kind == "proj":
                    emit_proj(g[1], g[2])

            filler_q = []

            def fill():
                while filler_q and cred["pe"] < cred["act"]:
                    emit_group(filler_q.pop(0))

            # filler order: deadline-sorted supply of PE work
            for kt in range(2, 8):
                filler_q.append(("v", kt))
            filler_q.append(("qk", 2, 0))    # mC t0 (needed by C(0))
            filler_q.append(("k2s", 0))
            filler_q += [("qk", 1, 1), ("qk", 0, 1)]
            for kt in range(8, 12):
                filler_q.append(("v", kt))
            filler_q += [("qk", 2, 1), ("k2s", 1)]
            filler_q += [("qk", 1, 2), ("qk", 0, 2)]
            for kt in range(12, 16):
                filler_q.append(("v", kt))
            filler_q += [("qk", 2, 2), ("k2s", 2)]
            filler_q += [("qk", 1, 3), ("qk", 0, 3)]
            filler_q += [("qk", 2, 3), ("k2s", 3)]

            def need(groups):
                for g in groups:
                    if g not in emitted:
                        if g in filler_q:
                            filler_q.remove(g)
                        emit_group(g)

            def qlo_of(kt, t):
                dm = kt - 4 * t
                return 128 * dm if dm >= 0 else 0

            mask_eng = nc.vector if "m" in KSAFE else nc.gpsimd

            def emit_mask(pT, o):
                mask_eng.tensor_mul(
                    pT[:, o:o + 128], pT[:, o:o + 128], trimask[:, :],
                )

            def emit_norm(h, t, py):
                ydest, yrow = (yA, 0) if h == 0 else (yA, 64) if h == 1 else (yB, 0)
                if "r" in KSAFE:
                    den = sbp.tile([1, QC], F32, tag="rec", name="rec")
                    nc.vector.tensor_copy(den[:, :], py[64:65, :])
                    bc = sbp.tile([64, QC], F32, tag="bc", name="bc")
                    nc.gpsimd.partition_broadcast(bc[:, :], den[0:1, :])
                    rec64 = sbp.tile([64, QC], F32, tag="rec64", name="rec64")
                    nc.vector.reciprocal(rec64[:, :], bc[:, :])
                    nc.vector.tensor_mul(
                        ydest[yrow:yrow + 64, t * QC:(t + 1) * QC],
                        py[0:64, :], rec64[:, :],
                    )
                    return
                rec = sbp.tile([1, QC], F32, tag="rec", name="rec")
                nc.vector.reciprocal_approx_fast(rec[:, :], py[64:65, :])
                bc = sbp.tile([64, QC], F32, tag="bc", name="bc")
                nc.gpsimd.partition_broadcast(bc[:, :], rec[0:1, :])
                nc.vector.tensor_mul(
                    ydest[yrow:yrow + 64, t * QC:(t + 1) * QC],
                    py[0:64, :], bc[:, :],
                )

            # ---- h01 attention block: heads 0,1 row-paired per kt ----
            def attn01(t):
                n_k = 4 * (t + 1)
                py0 = psY.tile([128, QC], F32, tag="py", name="py")
                py1 = psY.tile([128, QC], F32, tag="py", name="py")
                pend = None  # (kt, pT)
                for kt in range(n_k):
                    qlo = qlo_of(kt, t)
                    qg = t * QC
                    # S halves always write the full 512 cols (cols below
                    # qlo are masked-region scores that exp/PV never read)
                    # so the exp span is fully initialized by this tile.
                    ps = psS.tile([128, 2 * QC], F32, tag="ps", name="ps")
                    for half, (klo, khi) in enumerate(((0, 64), (64, 128))):
                        nc.tensor.matmul(
                            ps[:, half * QC:(half + 1) * QC],
                            lhsT=kA[klo:khi, kt * KT:(kt + 1) * KT],
                            rhs=qA[klo:khi, qg:qg + QC],
                            start=True, stop=True,
                        )
                    cred["pe"] += 230
                    pT = sbp.tile([128, 2 * QC], BF16, tag="pT", name="pT")
                    nc.scalar.activation(
                        pT[:, qlo:2 * QC], ps[:, qlo:2 * QC], AF.Exp,
                        scale=SCALE,
                    )
                    cred["act"] += (2 * QC - qlo + 352) / 1.2
                    if kt - 4 * t >= 0:
                        for half in range(2):
                            emit_mask(pT, half * QC + qlo)
                    if pend is not None:
                        pkt, ppT = pend
                        pqlo = qlo_of(pkt, t)
                        for h, half in ((0, 0), (1, 1)):
                            nc.tensor.matmul(
                                (py0 if h == 0 else py1)[0:65, pqlo:QC],
                                lhsT=vaug[:, pkt, h * 65:(h + 1) * 65],
                                rhs=ppT[:, half * QC + pqlo:(half + 1) * QC],
                                start=(pkt == 0), stop=(pkt == n_k - 1),
                            )
                        cred["pe"] += 2 * (QC - pqlo) / 2.4
                    pend = (kt, pT)
                    fill()
                pkt, ppT = pend
                pqlo = qlo_of(pkt, t)
                for h, half in ((0, 0), (1, 1)):
                    nc.tensor.matmul(
                        (py0 if h == 0 else py1)[0:65, pqlo:QC],
                        lhsT=vaug[:, pkt, h * 65:(h + 1) * 65],
                        rhs=ppT[:, half * QC + pqlo:(half + 1) * QC],
                        start=(pkt == 0), stop=(pkt == n_k - 1),
                    )
                cred["pe"] += 2 * (QC - pqlo) / 2.4
                emit_norm(0, t, py0)
                emit_norm(1, t, py1)

            # ---- h2 attention block: kt-pairs share one ps tile ----
            def attn2(t):
                n_k = 4 * (t + 1)
                py2 = psY.tile([128, QC], F32, tag="py", name="py")
                pend = None
                for p in range(n_k // 2):
                    kts = (2 * p, 2 * p + 1)
                    qg = t * QC
                    ps = psS.tile([128, 2 * QC], F32, tag="ps", name="ps")
                    for half, kt in enumerate(kts):
                        nc.tensor.matmul(
                            ps[:, half * QC:(half + 1) * QC],
                            lhsT=k2s[0:64, kt * KT:(kt + 1) * KT],
                            rhs=mCs[0:64, qg:qg + QC],
                            start=True, stop=True,
                        )
                        cred["pe"] += QC / 2.4
                    lo = qlo_of(kts[0], t)
                    pT = sbp.tile([128, 2 * QC], BF16, tag="pT", name="pT")
                    nc.scalar.activation(
                        pT[:, lo:2 * QC], ps[:, lo:2 * QC], AF.Exp,
                        scale=SCALE,
                    )
                    cred["act"] += (2 * QC - lo + 352) / 1.2
                    for half, kt in enumerate(kts):
                        if kt - 4 * t >= 0:
                            emit_mask(pT, half * QC + qlo_of(kt, t))
                    if pend is not None:
                        for pkt, ppT, phalf in pend:
                            pqlo = qlo_of(pkt, t)
                            nc.tensor.matmul(
                                py2[0:65, pqlo:QC],
                                lhsT=vaug[:, pkt, 2 * 65:3 * 65],
                                rhs=ppT[:, phalf * QC + pqlo:(phalf + 1) * QC],
                                start=(pkt == 0), stop=(pkt == n_k - 1),
                            )
                            cred["pe"] += (QC - pqlo) / 2.4
                    pend = [(kts[0], pT, 0), (kts[1], pT, 1)]
                    fill()
                for pkt, ppT, phalf in pend:
                    pqlo = qlo_of(pkt, t)
                    nc.tensor.matmul(
                        py2[0:65, pqlo:QC],
                        lhsT=vaug[:, pkt, 2 * 65:3 * 65],
                        rhs=ppT[:, phalf * QC + pqlo:(phalf + 1) * QC],
                        start=(pkt == 0), stop=(pkt == n_k - 1),
                    )
                    cred["pe"] += (QC - pqlo) / 2.4
                emit_norm(2, t, py2)

            # ---- schedule ----
            need([("qk", 0, 0), ("qk", 1, 0), ("v", 0), ("v", 1)])
            for t in range(NQC):
                need([("qk", 0, t), ("qk", 1, t)]
                     + [("v", kt) for kt in range(4 * t, 4 * t + 4)])
                attn01(t)
                need([("qk", 2, t), ("k2s", t)])
                attn2(t)
                for ct in range(NCH):
                    filler_q.append(("proj", ct, t))
            # flush whatever the filler scheduler didn't consume; the tail
            # projs alternate their PSUM->SBUF copy onto ACT (idle by now)
            rest = [g for g in filler_q if g not in emitted]
            for g in rest:
                if g[0] != "proj":
                    emit_group(g)
            projs = [g for g in rest if g[0] == "proj"]
            for i, g in enumerate(projs):
                emitted.add(g)
                emit_proj(g[1], g[2], on_act=(i % 2 == 1))

    nc.finalize()
    return nc


def kernel(x, W_attn, b_attn, W_proj, b_proj):
    global LAST_RESULTS
    B = x.shape[0]
    x = np.asarray(x, np.float32)
    W_attn = np.asarray(W_attn, np.float32)
    b_attn = np.asarray(b_attn, np.float32)
    W_proj = np.asarray(W_proj, np.float32)
    b_proj = np.asarray(b_proj, np.float32)

    if "nc" not in _CACHE:
        _CACHE["nc"] = build()
    nc = _CACHE["nc"]

    in_maps = []
    for c in range(8):
        b, g = divmod(c, 4)
        heads = [3 * g + i for i in range(HPC)]
        h0, h1, h2 = heads
        Q = lambda h: W_attn[:, 64 * h:64 * h + 64]
        K = lambda h: W_attn[:, C + 64 * h:C + 64 * h + 64]
        V = lambda h: W_attn[:, 2 * C + 64 * h:2 * C + 64 * h + 64]
        bQ = lambda h: b_attn[64 * h:64 * h + 64]
        bK = lambda h: b_attn[C + 64 * h:C + 64 * h + 64]
        bV = lambda h: b_attn[2 * C + 64 * h:2 * C + 64 * h + 64]
        # m-tiles: [Q0|Q1], [K0|K1], [Q2|K2]
        wqk = np.ascontiguousarray(np.concatenate(
            [Q(h0), Q(h1), K(h0), K(h1), Q(h2), K(h2)], 1)
        ).astype(ml_dtypes.bfloat16)
        wv = np.ascontiguousarray(np.concatenate(
            [V(h0), V(h1), V(h2)], 1)).astype(ml_dtypes.bfloat16)
        bcols = [bQ(h0), bQ(h1), bK(h0), bK(h1), bQ(h2), bK(h2)]
        bvec = np.concatenate(bcols)                      # [384] = 3 x 128
        battn = np.ascontiguousarray(bvec.reshape(3, 128).T)  # [128, 3]
        bvv = np.concatenate([bV(h0), bV(h1), bV(h2)])[None, :]
        wp0 = np.concatenate(
            [W_proj[64 * h:64 * h + 64, :] for h in (h0, h1)], 0)
        wp1 = W_proj[64 * h2:64 * h2 + 64, :]
        xt = np.ascontiguousarray(x[b].T)
        in_maps.append({
            "xTb": xt.astype(ml_dtypes.bfloat16),
            "wqk": wqk,
            "wv": wv,
            "battn": battn,
            "bv": bvv.astype(ml_dtypes.bfloat16),
            "wp0": np.ascontiguousarray(wp0).astype(ml_dtypes.bfloat16),
            "wp1": np.ascontiguousarray(wp1).astype(ml_dtypes.bfloat16),
        })

    res = run_bass_kernel_spmd(nc, in_maps, core_ids=list(range(8)))
    LAST_RESULTS = res

    out = np.zeros((B, T, C), np.float32)
    for c in range(8):
        b = c // 4
        out[b] += res.results[c]["yT"].T
    out += b_proj
    return out


# revision 24
# speedup vs baseline: 1.1517x; 1.1517x over previous
"""Causal self-attention (B=2, T=2048, C=768, H=12) on 8 TRN2 NeuronCores.

Sharding: core c -> batch b = c//4, head-group g = c%4 (heads 3g..3g+2).
Each core computes QKV for its 3 heads, causal attention, and a partial
c_proj (its heads' rows of W_proj). Host sums the 4 partials per batch.

Layout: feature-on-partitions for Q/K (S^T tiles [k, q]), but V is
computed directly in [t, d] layout (lhsT = x^T chunks), which feeds the
PV matmul with no PE transposes. Softmax over k uses an appended
ones-column on V so the PV accumulation yields [y^T; denom] in one
group. No max-subtraction: scores ~N(0,1), exp is fp32-safe.

qk m-tiles (host packs): m0=[Q0|Q1] m1=[K0|K1] m2=[Q2|K2], 128 rows
each so Q_h/K_h of heads 0/1 sit at partition bases 0/64. S matmuls for
h0 (rows 0:64) and h1 (rows 64:128) are emitted back-to-back and run
CONCURRENTLY in different PE row-groups (tile_position auto-derived),
halving S wall time. h2's K2 is copied to a base-0 scratch by GpSimd.

Engine budget per core (est): ACT ~69us (exp, the bottleneck), PE ~71us,
DVE ~50us, GpSimd ~35us. Emission software-pipelines one kt-step ahead
(S(kt+1) before PV(kt)) and a credit-based filler scheduler injects
QKV / V / proj matmul groups between attention steps so PE stays dense
while ACT runs exp. Reciprocal uses reciprocal_approx_fast (5x faster
than DVE reciprocal) on the [1,512] denom row, broadcast by GpSimd.
"""

import os

import numpy as np
import ml_dtypes

import concourse.bass as bass
import concourse.mybir as mybir
import concourse.tile as tile
from concourse import bacc
from concourse.bass_utils import run_bass_kernel_spmd
from concourse.masks import make_identity, make_upper_triangular

F32 = mybir.dt.float32
F32R = mybir.dt.float32r
BF16 = mybir.dt.bfloat16
AF = mybir.ActivationFunctionType

T = 2048           # sequence length
C = 768            # embed dim
HPC = 3            # heads per core
D = 64             # head dim
QC = 512           # q-chunk (psum bank width in fp32)
KT = 128           # k-tile
NKT = T // KT      # 16
NQC = T // QC      # 4
NCH = C // 128     # 6 contraction chunks
SCALE = 1.0 / 8.0  # 1/sqrt(64)

_CACHE = {}
LAST_RESULTS = None
_TCNT = [0]
# debug bisect switches: 'r' safe reciprocal, 'm' masks on vector,
# 'b' skip V-bias matmul
KSAFE = os.environ.get("KSAFE", "")


def mk_persist(pool, shape, dtype, name=None):
    if name is None:
        _TCNT[0] += 1
        name = f"pt{_TCNT[0]}"
    return pool.tile(shape, dtype, name=name, tag=name)


def build():
    nc = bacc.Bacc("TRN2", target_bir_lowering=False)

    xTb = nc.dram_tensor("xTb", [C, T], BF16, kind="ExternalInput")
    wqk = nc.dram_tensor("wqk", [C, 384], BF16, kind="ExternalInput")
    wv = nc.dram_tensor("wv", [C, 192], BF16, kind="ExternalInput")
    battn = nc.dram_tensor("battn", [128, 3], F32, kind="ExternalInput")
    bv = nc.dram_tensor("bv", [1, 192], BF16, kind="ExternalInput")
    wp0 = nc.dram_tensor("wp0", [128, C], BF16, kind="ExternalInput")
    wp1 = nc.dram_tensor("wp1", [64, C], BF16, kind="ExternalInput")
    yT = nc.dram_tensor("yT", [C, T], F32, kind="ExternalOutput")

    with tile.TileContext(nc) as tc, \
            tc.tile_pool(name="persist", bufs=1) as pp:
        # ---- persistent SBUF tensors ----
        xsbb = mk_persist(pp, [128, NCH, T], BF16)     # x^T bf16
        wqk_sb = mk_persist(pp, [128, NCH, 384], BF16)
        wv_sb = mk_persist(pp, [128, NCH, 192], BF16)
        battn_sb = mk_persist(pp, [128, 3], F32)
        bv_sb = mk_persist(pp, [1, 192], BF16)
        ones_r = mk_persist(pp, [1, 128], BF16)        # bias-matmul lhsT
        qA = mk_persist(pp, [128, T], F32R)    # [Q0|Q1]
        kA = mk_persist(pp, [128, T], F32R)    # [K0|K1]
        mCs = mk_persist(pp, [128, T], F32R)   # [Q2|K2]
        k2s = mk_persist(pp, [128, T], F32R)   # [K2|Q2] (swapped via DMA)
        vaug = mk_persist(pp, [128, NKT, 3 * 65], BF16)  # V[t,d]+ones per kt
        yA = mk_persist(pp, [128, T], BF16)    # y^T heads 0,1
        yB = mk_persist(pp, [64, T], BF16)     # y^T head 2
        wp0_sb = mk_persist(pp, [128, C], BF16)
        wp1_sb = mk_persist(pp, [64, C], BF16)
        trimask_s = mk_persist(pp, [128, 128], F32)   # [k,q]=1 iff k<=q
        trimask = mk_persist(pp, [128, 128], BF16)
        ident_s = mk_persist(pp, [128, 128], F32)
        ident = mk_persist(pp, [128, 128], F32R)
        maskM_s = mk_persist(pp, [128, 128], F32)  # -1e9 where k>q else 0
        maskM = mk_persist(pp, [128, 128], F32R)
        dummy = mk_persist(pp, [1, 8], F32)

        make_upper_triangular(nc, trimask_s[:, :], val=1.0, diag=True)
        nc.vector.tensor_copy(trimask[:, :], trimask_s[:, :])
        make_identity(nc, ident_s[:, :])
        nc.vector.tensor_copy(ident[:, :], ident_s[:, :])
        # maskM = (trimask - 1) * 1e9 : 0 on k<=q, -1e9 on k>q
        nc.vector.tensor_scalar(
            maskM_s[:, :], trimask_s[:, :], -1.0, 1e9,
            op0=mybir.AluOpType.add, op1=mybir.AluOpType.mult,
        )
        nc.vector.tensor_copy(maskM[:, :], maskM_s[:, :])
        nc.vector.memset(ones_r[:, :], 1.0)
        # preload the exp table set while DMAs run
        nc.vector.memset(dummy[:, :], 0.0)
        nc.scalar.activation(dummy[:, :], dummy[:, :], AF.Exp)
        for i in range(3):  # ones columns of vaug
            nc.vector.memset(vaug[:, :, 65 * i + 64:65 * i + 65], 1.0)

        # ---- DMA loads, earliest-needed first ----
        for cc in range(NCH):
            nc.sync.dma_start(wqk_sb[:, cc, 0:256], wqk[cc * 128:(cc + 1) * 128, 0:256])
        for cc in range(NCH):
            nc.sync.dma_start(xsbb[:, cc, 0:QC], xTb[cc * 128:(cc + 1) * 128, 0:QC])
        nc.sync.dma_start(battn_sb[:, :], battn[:, :])
        nc.sync.dma_start(bv_sb[:, :], bv[:, :])
        for cc in range(NCH):
            nc.sync.dma_start(wv_sb[:, cc, :], wv[cc * 128:(cc + 1) * 128, :])
            nc.sync.dma_start(wqk_sb[:, cc, 256:384], wqk[cc * 128:(cc + 1) * 128, 256:384])
        for t in range(1, NQC):
            for cc in range(NCH):
                nc.sync.dma_start(
                    xsbb[:, cc, t * QC:(t + 1) * QC],
                    xTb[cc * 128:(cc + 1) * 128, t * QC:(t + 1) * QC])
        nc.sync.dma_start(wp0_sb[:, :], wp0[:, :])
        nc.sync.dma_start(wp1_sb[:, :], wp1[:, :])

        qk_dest = [qA, kA, mCs]

        with (
            tc.tile_pool(name="psS", bufs=2, space="PSUM") as psS,
            tc.tile_pool(name="psY", bufs=2, space="PSUM") as psY,
            tc.tile_pool(name="psM", bufs=2, space="PSUM") as psM,
            tc.tile_pool(name="sb", bufs=8) as sbp,
        ):
            # --- emit helpers; each returns nothing, updates credit ---
            cred = {"pe": 0.0, "act": 0.0}

            def emit_qk(m, t):
                ps = psM.tile([128, QC], F32, tag="pm", name="pm")
                for cc in range(NCH):
                    nc.tensor.matmul(
                        ps[:, :],
                        lhsT=wqk_sb[:, cc, m * 128:(m + 1) * 128],
                        rhs=xsbb[:, cc, t * QC:(t + 1) * QC],
                        start=(cc == 0), stop=(cc == NCH - 1),
                    )
                nc.vector.tensor_scalar_add(
                    qk_dest[m][:, t * QC:(t + 1) * QC], ps[:, :],
                    battn_sb[:, m:m + 1],
                )
                cred["pe"] += 1350

            def emit_k2s(t):
                # partition-swapped copy of mC ([Q2|K2] -> [K2|Q2]): only
                # DMA can move data across partitions cheaply. The swapped
                # tile lets h2's S matmuls run row-paired like h0/h1.
                nc.sync.dma_start(
                    k2s[0:64, t * QC:(t + 1) * QC],
                    mCs[64:128, t * QC:(t + 1) * QC],
                )
                nc.sync.dma_start(
                    k2s[64:128, t * QC:(t + 1) * QC],
                    mCs[0:64, t * QC:(t + 1) * QC],
                )

            def emit_v(kt):
                ps = psM.tile([128, QC], F32, tag="pm", name="pm")
                skip_bias = "b" in KSAFE
                for cc in range(NCH):
                    nc.tensor.matmul(
                        ps[:, 0:192],
                        lhsT=xsbb[:, cc, kt * KT:(kt + 1) * KT],
                        rhs=wv_sb[:, cc, :],
                        start=(cc == 0), stop=(skip_bias and cc == NCH - 1),
                    )
                if not skip_bias:
                    nc.tensor.matmul(
                        ps[:, 0:192],
                        lhsT=ones_r[:, :],
                        rhs=bv_sb[:, :],
                        start=False, stop=True,
                    )
                nc.vector.tensor_copy(
                    vaug[:, kt, :].rearrange("p (a b) -> p a b", b=65)[:, :, 0:64],
                    ps[:, 0:192].rearrange("p (a b) -> p a b", b=64),
                )
                cred["pe"] += 640

            def emit_proj(ct, t, on_act=False):
                ps = psM.tile([128, QC], F32, tag="pm", name="pm")
                nc.tensor.matmul(
                    ps[:, :],
                    lhsT=wp0_sb[:, ct * 128:(ct + 1) * 128],
                    rhs=yA[:, t * QC:(t + 1) * QC],
                    start=True, stop=False,
                )
                nc.tensor.matmul(
                    ps[:, :],
                    lhsT=wp1_sb[0:64, ct * 128:(ct + 1) * 128],
                    rhs=yB[0:64, t * QC:(t + 1) * QC],
                    start=False, stop=True,
                )
                osb = sbp.tile([128, QC], F32, tag="osb", name="osb")
                if on_act:
                    nc.scalar.activation(osb[:, :], ps[:, :], AF.Copy)
                else:
                    nc.vector.tensor_copy(osb[:, :], ps[:, :])
                nc.sync.dma_start(
                    yT[ct * 128:(ct + 1) * 128, t * QC:(t + 1) * QC],
                    osb[:, :],
                )
                cred["pe"] += 500

            # ---- filler scheduler ----
            emitted = set()

            def emit_group(g):
                if g in emitted:
                    return
                emitted.add(g)
                kind = g[0]
                if kind == "qk":
                    emit_qk(g[1], g[2])
                elif kind == "v":
                    emit_v(g[1])
                elif kind == "k2s":
                    emit_k2s(g[1])
                elif kind == "proj":
                    emit_proj(g[1], g[2])

            filler_q = []

            def fill():
                while filler_q and cred["pe"] < cred["act"]:
                    emit_group(filler_q.pop(0))

            # filler order: deadline-sorted supply of PE work
            for kt in range(2, 8):
                filler_q.append(("v", kt))
            filler_q.append(("qk", 2, 0))    # mC t0 (needed by C(0))
            filler_q.append(("k2s", 0))
            filler_q += [("qk", 1, 1), ("qk", 0, 1)]
            for kt in range(8, 12):
                filler_q.append(("v", kt))
            filler_q += [("qk", 2, 1), ("k2s", 1)]
            filler_q += [("qk", 1, 2), ("qk", 0, 2)]
            for kt in range(12, 16):
                filler_q.append(("v", kt))
            filler_q += [("qk", 2, 2), ("k2s", 2)]
            filler_q += [("qk", 1, 3), ("qk", 0, 3)]
            filler_q += [("qk", 2, 3), ("k2s", 3)]

            def need(groups):
                for g in groups:
                    if g not in emitted:
                        if g in filler_q:
                            filler_q.remove(g)
                        emit_group(g)

            def qlo_of(kt, t):
                dm = kt - 4 * t
                return 128 * dm if dm >= 0 else 0

            def emit_mask(pT, o):
                nc.vector.tensor_mul(
                    pT[:, o:o + 128], pT[:, o:o + 128], trimask[:, :],
                )

            def emit_norm(h, t, py):
                ydest, yrow = (yA, 0) if h == 0 else (yA, 64) if h == 1 else (yB, 0)
                if "r" in KSAFE:
                    den = sbp.tile([1, QC], F32, tag="rec", name="rec")
                    nc.vector.tensor_copy(den[:, :], py[64:65, :])
                    bc = sbp.tile([64, QC], F32, tag="bc", name="bc")
                    nc.gpsimd.partition_broadcast(bc[:, :], den[0:1, :])
                    rec64 = sbp.tile([64, QC], F32, tag="rec64", name="rec64")
                    nc.vector.reciprocal(rec64[:, :], bc[:, :])
                    nc.vector.tensor_mul(
                        ydest[yrow:yrow + 64, t * QC:(t + 1) * QC],
                        py[0:64, :], rec64[:, :],
                    )
                    return
                # reciprocal_approx_fast mis-reads PSUM at partition offset
                # 64 on HW (unit-tested) -> stage the denom row to SBUF
                # partition 0 first (baseline-proven DVE row move).
                den = sbp.tile([1, QC], F32, tag="den", name="den")
                nc.vector.tensor_copy(den[:, :], py[64:65, :])
                rec = sbp.tile([1, QC], F32, tag="rec", name="rec")
                nc.vector.reciprocal_approx_fast(rec[:, :], den[:, :])
                bc = sbp.tile([64, QC], F32, tag="bc", name="bc")
                nc.gpsimd.partition_broadcast(bc[:, :], rec[0:1, :])
                nc.vector.tensor_mul(
                    ydest[yrow:yrow + 64, t * QC:(t + 1) * QC],
                    py[0:64, :], bc[:, :],
                )

            # ---- h01 attention block: heads 0,1 row-paired per kt ----
            def attn01(t):
                n_k = 4 * (t + 1)
                py0 = psY.tile([128, QC], F32, tag="py", name="py")
                py1 = psY.tile([128, QC], F32, tag="py", name="py")
                pend = None  # (kt, pT)
                for kt in range(n_k):
                    qlo = qlo_of(kt, t)
                    qg = t * QC
                    # S halves always write the full 512 cols (cols below
                    # qlo are masked-region scores that exp/PV never read)
                    # so the exp span is fully initialized by this tile.
                    # On diagonal k-tiles a follow-up identity-matmul
                    # accumulates -1e9 onto the k>q half of the diagonal
                    # block, so exp yields exact zeros there (no mask mul).
                    diag = kt - 4 * t >= 0
                    ps = psS.tile([128, 2 * QC], F32, tag="ps", name="ps")
                    for half, (klo, khi) in enumerate(((0, 64), (64, 128))):
                        nc.tensor.matmul(
                            ps[:, half * QC:(half + 1) * QC],
                            lhsT=kA[klo:khi, kt * KT:(kt + 1) * KT],
                            rhs=qA[klo:khi, qg:qg + QC],
                            start=True, stop=not diag,
                        )
                    if diag:
                        for half in range(2):
                            o = half * QC + qlo
                            nc.tensor.matmul(
                                ps[:, o:o + 128],
                                lhsT=ident[:, :], rhs=maskM[:, :],
                                start=False, stop=True,
                            )
                        cred["pe"] += 160
                    cred["pe"] += 230
                    pT = sbp.tile([128, 2 * QC], BF16, tag="pT", name="pT")
                    nc.scalar.activation(
                        pT[:, qlo:2 * QC], ps[:, qlo:2 * QC], AF.Exp,
                        scale=SCALE,
                    )
                    cred["act"] += (2 * QC - qlo + 352) / 1.2
                    if pend is not None:
                        pkt, ppT = pend
                        pqlo = qlo_of(pkt, t)
                        for h, half in ((0, 0), (1, 1)):
                            nc.tensor.matmul(
                                (py0 if h == 0 else py1)[0:65, pqlo:QC],
                                lhsT=vaug[:, pkt, h * 65:(h + 1) * 65],
                                rhs=ppT[:, half * QC + pqlo:(half + 1) * QC],
                                start=(pkt == 0), stop=(pkt == n_k - 1),
                            )
                        cred["pe"] += 2 * (QC - pqlo) / 2.4
                    pend = (kt, pT)
                    fill()
                pkt, ppT = pend
                pqlo = qlo_of(pkt, t)
                for h, half in ((0, 0), (1, 1)):
                    nc.tensor.matmul(
                        (py0 if h == 0 else py1)[0:65, pqlo:QC],
                        lhsT=vaug[:, pkt, h * 65:(h + 1) * 65],
                        rhs=ppT[:, half * QC + pqlo:(half + 1) * QC],
                        start=(pkt == 0), stop=(pkt == n_k - 1),
                    )
                cred["pe"] += 2 * (QC - pqlo) / 2.4
                emit_norm(0, t, py0)
                emit_norm(1, t, py1)

            # ---- h2 attention block: kt-pairs share one ps tile ----
            def attn2(t):
                n_k = 4 * (t + 1)
                py2 = psY.tile([128, QC], F32, tag="py", name="py")
                pend = None
                for p in range(n_k // 2):
                    kts = (2 * p, 2 * p + 1)
                    qg = t * QC
                    # kt even runs on PE rows 0:63 (K2/Q2 at base 0), kt
                    # odd on rows 64:127 (base 64) -> concurrent row tiles
                    diag = kts[0] - 4 * t >= 0
                    ps = psS.tile([128, 2 * QC], F32, tag="ps", name="ps")
                    nc.tensor.matmul(
                        ps[:, 0:QC],
                        lhsT=k2s[0:64, kts[0] * KT:(kts[0] + 1) * KT],
                        rhs=mCs[0:64, qg:qg + QC],
                        start=True, stop=not diag,
                    )
                    nc.tensor.matmul(
                        ps[:, QC:2 * QC],
                        lhsT=mCs[64:128, kts[1] * KT:(kts[1] + 1) * KT],
                        rhs=k2s[64:128, qg:qg + QC],
                        start=True, stop=not diag,
                    )
                    if diag:
                        for half, kt in enumerate(kts):
                            o = half * QC + qlo_of(kt, t)
                            nc.tensor.matmul(
                                ps[:, o:o + 128],
                                lhsT=ident[:, :], rhs=maskM[:, :],
                                start=False, stop=True,
                            )
                        cred["pe"] += 160
                    cred["pe"] += 230
                    lo = qlo_of(kts[0], t)
                    pT = sbp.tile([128, 2 * QC], BF16, tag="pT", name="pT")
                    nc.scalar.activation(
                        pT[:, lo:2 * QC], ps[:, lo:2 * QC], AF.Exp,
                        scale=SCALE,
                    )
                    cred["act"] += (2 * QC - lo + 352) / 1.2
                    if pend is not None:
                        for pkt, ppT, phalf in pend:
                            pqlo = qlo_of(pkt, t)
                            nc.tensor.matmul(
                                py2[0:65, pqlo:QC],
                                lhsT=vaug[:, pkt, 2 * 65:3 * 65],
                                rhs=ppT[:, phalf * QC + pqlo:(phalf + 1) * QC],
                                start=(pkt == 0), stop=(pkt == n_k - 1),
                            )
                            cred["pe"] += (QC - pqlo) / 2.4
                    pend = [(kts[0], pT, 0), (kts[1], pT, 1)]
                    fill()
                for pkt, ppT, phalf in pend:
                    pqlo = qlo_of(pkt, t)
                    nc.tensor.matmul(
                        py2[0:65, pqlo:QC],
                        lhsT=vaug[:, pkt, 2 * 65:3 * 65],
                        rhs=ppT[:, phalf * QC + pqlo:(phalf + 1) * QC],
                        start=(pkt == 0), stop=(pkt == n_k - 1),
                    )
                    cred["pe"] += (QC - pqlo) / 2.4
                emit_norm(2, t, py2)

            # ---- schedule ----
            # pre-need one block ahead so block boundaries never leave the
            # PE idle long enough for HAM to re-throttle the clock.
            need([("qk", 0, 0), ("qk", 1, 0), ("v", 0), ("v", 1)])
            for t in range(NQC):
                need([("qk", 0, t), ("qk", 1, t)]
                     + [("v", kt) for kt in range(4 * t, 4 * t + 4)]
                     + [("qk", 2, t), ("k2s", t)])
                attn01(t)
                if t + 1 < NQC:
                    need([("qk", 0, t + 1), ("qk", 1, t + 1)]
                         + [("v", kt) for kt in range(4 * t + 4, 4 * t + 8)])
                attn2(t)
                for ct in range(NCH):
                    filler_q.append(("proj", ct, t))
            # flush whatever the filler scheduler didn't consume; the tail
            # projs alternate their PSUM->SBUF copy onto ACT (idle by now)
            rest = [g for g in filler_q if g not in emitted]
            for g in rest:
                if g[0] != "proj":
                    emit_group(g)
            projs = [g for g in rest if g[0] == "proj"]
            for i, g in enumerate(projs):
                emitted.add(g)
                emit_proj(g[1], g[2], on_act=(i % 2 == 1))

    nc.finalize()
    return nc


def kernel(x, W_attn, b_attn, W_proj, b_proj):
    global LAST_RESULTS
    B = x.shape[0]
    x = np.asarray(x, np.float32)
    W_attn = np.asarray(W_attn, np.float32)
    b_attn = np.asarray(b_attn, np.float32)
    W_proj = np.asarray(W_proj, np.float32)
    b_proj = np.asarray(b_proj, np.float32)

    if "nc" not in _CACHE:
        _CACHE["nc"] = build()
    nc = _CACHE["nc"]

    in_maps = []
    for c in range(8):
        b, g = divmod(c, 4)
        heads = [3 * g + i for i in range(HPC)]
        h0, h1, h2 = heads
        Q = lambda h: W_attn[:, 64 * h:64 * h + 64]
        K = lambda h: W_attn[:, C + 64 * h:C + 64 * h + 64]
        V = lambda h: W_attn[:, 2 * C + 64 * h:2 * C + 64 * h + 64]
        bQ = lambda h: b_attn[64 * h:64 * h + 64]
        bK = lambda h: b_attn[C + 64 * h:C + 64 * h + 64]
        bV = lambda h: b_attn[2 * C + 64 * h:2 * C + 64 * h + 64]
        # m-tiles: [Q0|Q1], [K0|K1], [Q2|K2]
        wqk = np.ascontiguousarray(np.concatenate(
            [Q(h0), Q(h1), K(h0), K(h1), Q(h2), K(h2)], 1)
        ).astype(ml_dtypes.bfloat16)
        wv = np.ascontiguousarray(np.concatenate(
            [V(h0), V(h1), V(h2)], 1)).astype(ml_dtypes.bfloat16)
        bcols = [bQ(h0), bQ(h1), bK(h0), bK(h1), bQ(h2), bK(h2)]
        bvec = np.concatenate(bcols)                      # [384] = 3 x 128
        battn = np.ascontiguousarray(bvec.reshape(3, 128).T)  # [128, 3]
        bvv = np.concatenate([bV(h0), bV(h1), bV(h2)])[None, :]
        wp0 = np.concatenate(
            [W_proj[64 * h:64 * h + 64, :] for h in (h0, h1)], 0)
        wp1 = W_proj[64 * h2:64 * h2 + 64, :]
        xt = np.ascontiguousarray(x[b].T)
        in_maps.append({
            "xTb": xt.astype(ml_dtypes.bfloat16),
            "wqk": wqk,
            "wv": wv,
            "battn": battn,
            "bv": bvv.astype(ml_dtypes.bfloat16),
            "wp0": np.ascontiguousarray(wp0).astype(ml_dtypes.bfloat16),
            "wp1": np.ascontiguousarray(wp1).astype(ml_dtypes.bfloat16),
        })

    res = run_bass_kernel_spmd(nc, in_maps, core_ids=list(range(8)))
    LAST_RESULTS = res

    out = np.zeros((B, T, C), np.float32)
    for c in range(8):
        b = c // 4
        out[b] += res.results[c]["yT"].T
    out += b_proj
    return out


# revision 31
# speedup vs baseline: 1.3522x; 1.1741x over previous
"""Causal self-attention (B=2, T=2048, C=768, H=12) on 8 TRN2 NeuronCores.

Sharding: core c -> batch b = c//4, head-group g = c%4 (heads 3g..3g+2).
Each core computes QKV for its 3 heads, causal attention, and a partial
c_proj (its heads' rows of W_proj). Host sums the 4 partials per batch.

Layout: feature-on-partitions for Q/K (S^T tiles [k, q]), but V is
computed directly in [t, d] layout (lhsT = x^T chunks), which feeds the
PV matmul with no PE transposes. Softmax over k uses an appended
ones-column on V so the PV accumulation yields [y^T; denom] in one
group. No max-subtraction: scores ~N(0,1), exp is fp32-safe.

qk m-tiles (host packs): m0=[Q0|Q1] m1=[K0|K1] m2=[Q2|K2], 128 rows
each so Q_h/K_h of heads 0/1 sit at partition bases 0/64. S matmuls for
h0 (rows 0:64) and h1 (rows 64:128) are emitted back-to-back and run
CONCURRENTLY in different PE row-groups (tile_position auto-derived),
halving S wall time. h2's K2 is copied to a base-0 scratch by GpSimd.

Engine budget per core (est): ACT ~69us (exp, the bottleneck), PE ~71us,
DVE ~50us, GpSimd ~35us. Emission software-pipelines one kt-step ahead
(S(kt+1) before PV(kt)) and a credit-based filler scheduler injects
QKV / V / proj matmul groups between attention steps so PE stays dense
while ACT runs exp. Reciprocal uses reciprocal_approx_fast (5x faster
than DVE reciprocal) on the [1,512] denom row, broadcast by GpSimd.
"""

import os

import numpy as np
import ml_dtypes

import concourse.bass as bass
import concourse.mybir as mybir
import concourse.tile as tile
from concourse import bacc
from concourse.bass_utils import run_bass_kernel_spmd
from concourse.masks import make_identity, make_upper_triangular

F32 = mybir.dt.float32
F32R = mybir.dt.float32r
BF16 = mybir.dt.bfloat16
AF = mybir.ActivationFunctionType

T = 2048           # sequence length
C = 768            # embed dim
HPC = 3            # heads per core
D = 64             # head dim
QC = 512           # q-chunk (psum bank width in fp32)
KT = 128           # k-tile
NKT = T // KT      # 16
NQC = T // QC      # 4
NCH = C // 128     # 6 contraction chunks
SCALE = 1.0 / 8.0  # 1/sqrt(64)

_CACHE = {}
LAST_RESULTS = None
_TCNT = [0]
# debug bisect switches: 'r' safe reciprocal, 'm' masks on vector,
# 'b' skip V-bias matmul
KSAFE = os.environ.get("KSAFE", "")


def mk_persist(pool, shape, dtype, name=None):
    if name is None:
        _TCNT[0] += 1
        name = f"pt{_TCNT[0]}"
    return pool.tile(shape, dtype, name=name, tag=name)


def build(vbias=True):
    nc = bacc.Bacc("TRN2", target_bir_lowering=False)

    xTb = nc.dram_tensor("xTb", [C, T], BF16, kind="ExternalInput")
    wqk = nc.dram_tensor("wqk", [C, 384], BF16, kind="ExternalInput")
    wv = nc.dram_tensor("wv", [C, 192], BF16, kind="ExternalInput")
    battn = nc.dram_tensor("battn", [128, 3], F32, kind="ExternalInput")
    bv = nc.dram_tensor("bv", [1, 192], BF16, kind="ExternalInput")
    wp0 = nc.dram_tensor("wp0", [128, C], BF16, kind="ExternalInput")
    wp1 = nc.dram_tensor("wp1", [64, C], BF16, kind="ExternalInput")
    yT = nc.dram_tensor("yT", [C, T], F32, kind="ExternalOutput")

    with tile.TileContext(nc) as tc, \
            tc.tile_pool(name="persist", bufs=1) as pp:
        # ---- persistent SBUF tensors ----
        xsbb = mk_persist(pp, [128, NCH, T], BF16)     # x^T bf16
        wqk_sb = mk_persist(pp, [128, NCH, 384], BF16)
        wv_sb = mk_persist(pp, [128, NCH, 192], BF16)
        battn_sb = mk_persist(pp, [128, 3], F32)
        bv_sb = mk_persist(pp, [1, 192], BF16)
        ones_r = mk_persist(pp, [1, 128], BF16)        # bias-matmul lhsT
        qA = mk_persist(pp, [128, T], F32R)    # [Q0|Q1]
        kA = mk_persist(pp, [128, T], F32R)    # [K0|K1]
        mCs = mk_persist(pp, [128, T], F32R)   # [Q2|K2]
        k2s = mk_persist(pp, [128, T], F32R)   # [K2|Q2] (swapped via DMA)
        vaug = mk_persist(pp, [128, NKT, 3 * 65], BF16)  # V[t,d]+ones per kt
        yA = mk_persist(pp, [128, T], BF16)    # y^T heads 0,1
        yB = mk_persist(pp, [64, T], BF16)     # y^T head 2
        wp0_sb = mk_persist(pp, [128, C], BF16)
        wp1_sb = mk_persist(pp, [64, C], BF16)
        trimask_s = mk_persist(pp, [128, 128], F32)   # [k,q]=1 iff k<=q
        ident_s = mk_persist(pp, [128, 128], F32)
        ident = mk_persist(pp, [128, 128], BF16)   # bf16 -> FWL ldweights
        maskM_s = mk_persist(pp, [128, 128], F32)  # -1e9 where k>q else 0
        maskM = mk_persist(pp, [128, 128], BF16)
        dummy = mk_persist(pp, [1, 8], F32)

        make_upper_triangular(nc, trimask_s[:, :], val=1.0, diag=True)
        make_identity(nc, ident_s[:, :])
        nc.vector.tensor_copy(ident[:, :], ident_s[:, :])
        # maskM = (trimask - 1) * 1e9 : 0 on k<=q, -1e9 on k>q
        nc.vector.tensor_scalar(
            maskM_s[:, :], trimask_s[:, :], -1.0, 1e9,
            op0=mybir.AluOpType.add, op1=mybir.AluOpType.mult,
        )
        nc.vector.tensor_copy(maskM[:, :], maskM_s[:, :])
        nc.vector.memset(ones_r[:, :], 1.0)
        # preload the exp table set while DMAs run
        nc.vector.memset(dummy[:, :], 0.0)
        nc.scalar.activation(dummy[:, :], dummy[:, :], AF.Exp)
        for i in range(3):  # ones columns of vaug
            nc.vector.memset(vaug[:, :, 65 * i + 64:65 * i + 65], 1.0)

        # ---- DMA loads, earliest-needed first, coalesced into few
        # dispatches (each dma_start costs ~600ns of queue dispatch) ----
        nc.sync.dma_start(
            wqk_sb[:, :, 0:256],
            wqk[:, 0:256].rearrange("(c p) m -> p c m", p=128))
        nc.sync.dma_start(
            wv_sb[:, :, :], wv[:, :].rearrange("(c p) m -> p c m", p=128))
        nc.sync.dma_start(
            xsbb[:, :, 0:QC],
            xTb[:, 0:QC].rearrange("(c p) t -> p c t", p=128))
        nc.sync.dma_start(battn_sb[:, :], battn[:, :])
        nc.sync.dma_start(bv_sb[:, :], bv[:, :])
        nc.sync.dma_start(
            wqk_sb[:, :, 256:384],
            wqk[:, 256:384].rearrange("(c p) m -> p c m", p=128))
        nc.sync.dma_start(
            xsbb[:, :, QC:2 * QC],
            xTb[:, QC:2 * QC].rearrange("(c p) t -> p c t", p=128))

        def dma_late_loads():
            # issued from inside the schedule so mid-stream DMAs (k2s,
            # output stores) aren't stuck behind these dispatches
            nc.sync.dma_start(
                xsbb[:, :, 2 * QC:T],
                xTb[:, 2 * QC:T].rearrange("(c p) t -> p c t", p=128))
            nc.sync.dma_start(wp0_sb[:, :], wp0[:, :])
            nc.sync.dma_start(wp1_sb[:, :], wp1[:, :])

        qk_dest = [qA, kA, mCs]

        with (
            tc.tile_pool(name="psS", bufs=2, space="PSUM") as psS,
            tc.tile_pool(name="psY", bufs=2, space="PSUM") as psY,
            tc.tile_pool(name="psM", bufs=2, space="PSUM") as psM,
            tc.tile_pool(name="sb", bufs=8) as sbp,
        ):
            # --- emit helpers; each returns nothing, updates credit ---
            cred = {"pe": 0.0, "act": 0.0}

            def emit_qk(m, t):
                ps = psM.tile([128, QC], F32, tag="pm", name="pm")
                for cc in range(NCH):
                    nc.tensor.matmul(
                        ps[:, :],
                        lhsT=wqk_sb[:, cc, m * 128:(m + 1) * 128],
                        rhs=xsbb[:, cc, t * QC:(t + 1) * QC],
                        start=(cc == 0), stop=(cc == NCH - 1),
                    )
                nc.vector.tensor_scalar_add(
                    qk_dest[m][:, t * QC:(t + 1) * QC], ps[:, :],
                    battn_sb[:, m:m + 1],
                )
                cred["pe"] += 1350

            def emit_k2s(t):
                # partition-swapped copy of mC ([Q2|K2] -> [K2|Q2]): only
                # DMA can move data across partitions cheaply. The swapped
                # tile lets h2's S matmuls run row-paired like h0/h1.
                nc.sync.dma_start(
                    k2s[0:64, t * QC:(t + 1) * QC],
                    mCs[64:128, t * QC:(t + 1) * QC],
                )
                nc.sync.dma_start(
                    k2s[64:128, t * QC:(t + 1) * QC],
                    mCs[0:64, t * QC:(t + 1) * QC],
                )

            def emit_v(kt):
                ps = psM.tile([128, QC], F32, tag="pm", name="pm")
                skip_bias = not vbias or "b" in KSAFE
                for cc in range(NCH):
                    nc.tensor.matmul(
                        ps[:, 0:192],
                        lhsT=xsbb[:, cc, kt * KT:(kt + 1) * KT],
                        rhs=wv_sb[:, cc, :],
                        start=(cc == 0), stop=(skip_bias and cc == NCH - 1),
                    )
                if not skip_bias:
                    nc.tensor.matmul(
                        ps[:, 0:192],
                        lhsT=ones_r[:, :],
                        rhs=bv_sb[:, :],
                        start=False, stop=True,
                    )
                nc.vector.tensor_copy(
                    vaug[:, kt, :].rearrange("p (a b) -> p a b", b=65)[:, :, 0:64],
                    ps[:, 0:192].rearrange("p (a b) -> p a b", b=64),
                )
                cred["pe"] += 640

            def emit_proj(ct, t, on_act=False):
                ps = psM.tile([128, QC], F32, tag="pm", name="pm")
                nc.tensor.matmul(
                    ps[:, :],
                    lhsT=wp0_sb[:, ct * 128:(ct + 1) * 128],
                    rhs=yA[:, t * QC:(t + 1) * QC],
                    start=True, stop=False,
                )
                nc.tensor.matmul(
                    ps[:, :],
                    lhsT=wp1_sb[0:64, ct * 128:(ct + 1) * 128],
                    rhs=yB[0:64, t * QC:(t + 1) * QC],
                    start=False, stop=True,
                )
                osb = sbp.tile([128, QC], F32, tag="osb", name="osb")
                if on_act:
                    nc.scalar.activation(osb[:, :], ps[:, :], AF.Copy)
                else:
                    nc.vector.tensor_copy(osb[:, :], ps[:, :])
                nc.sync.dma_start(
                    yT[ct * 128:(ct + 1) * 128, t * QC:(t + 1) * QC],
                    osb[:, :],
                )
                cred["pe"] += 500

            # ---- filler scheduler ----
            emitted = set()

            def emit_group(g):
                if g in emitted:
                    return
                emitted.add(g)
                kind = g[0]
                if kind == "qk":
                    emit_qk(g[1], g[2])
                elif kind == "v":
                    emit_v(g[1])
                elif kind == "k2s":
                    emit_k2s(g[1])
                elif kind == "proj":
                    emit_proj(g[1], g[2])

            filler_q = []

            def fill():
                while filler_q and cred["pe"] < cred["act"]:
                    emit_group(filler_q.pop(0))

            # filler order: deadline-sorted supply of PE work
            for kt in range(2, 8):
                filler_q.append(("v", kt))
            filler_q.append(("qk", 2, 0))    # mC t0 (needed by C(0))
            filler_q.append(("k2s", 0))
            filler_q += [("qk", 1, 1), ("qk", 0, 1)]
            for kt in range(8, 12):
                filler_q.append(("v", kt))
            filler_q += [("qk", 2, 1), ("k2s", 1)]
            filler_q += [("qk", 1, 2), ("qk", 0, 2)]
            for kt in range(12, 16):
                filler_q.append(("v", kt))
            filler_q += [("qk", 2, 2), ("k2s", 2)]
            filler_q += [("qk", 1, 3), ("qk", 0, 3)]
            filler_q += [("qk", 2, 3), ("k2s", 3)]

            def need(groups):
                for g in groups:
                    if g not in emitted:
                        if g in filler_q:
                            filler_q.remove(g)
                        emit_group(g)

            def qlo_of(kt, t):
                dm = kt - 4 * t
                return 128 * dm if dm >= 0 else 0

            def emit_norm(h, t, py):
                ydest, yrow = (yA, 0) if h == 0 else (yA, 64) if h == 1 else (yB, 0)
                if "r" in KSAFE:
                    den = sbp.tile([1, QC], F32, tag="rec", name="rec")
                    nc.vector.tensor_copy(den[:, :], py[64:65, :])
                    bc = sbp.tile([64, QC], F32, tag="bc", name="bc")
                    nc.gpsimd.partition_broadcast(bc[:, :], den[0:1, :])
                    rec64 = sbp.tile([64, QC], F32, tag="rec64", name="rec64")
                    nc.vector.reciprocal(rec64[:, :], bc[:, :])
                    nc.vector.tensor_mul(
                        ydest[yrow:yrow + 64, t * QC:(t + 1) * QC],
                        py[0:64, :], rec64[:, :],
                    )
                    return
                # reciprocal_approx_fast mis-reads PSUM at partition offset
                # 64 on HW (unit-tested) -> stage the denom row to SBUF
                # partition 0 first (baseline-proven DVE row move).
                den = sbp.tile([1, QC], F32, tag="den", name="den")
                nc.vector.tensor_copy(den[:, :], py[64:65, :])
                rec = sbp.tile([1, QC], F32, tag="rec", name="rec")
                nc.vector.reciprocal_approx_fast(rec[:, :], den[:, :])
                bc = sbp.tile([64, QC], F32, tag="bc", name="bc")
                nc.gpsimd.partition_broadcast(bc[:, :], rec[0:1, :])
                nc.vector.tensor_mul(
                    ydest[yrow:yrow + 64, t * QC:(t + 1) * QC],
                    py[0:64, :], bc[:, :],
                )

            # ---- h01 attention block: heads 0,1 row-paired per kt ----
            def attn01(t):
                n_k = 4 * (t + 1)
                py0 = psY.tile([128, QC], F32, tag="py", name="py")
                py1 = psY.tile([128, QC], F32, tag="py", name="py")
                pend = None  # (kt, pT)
                for kt in range(n_k):
                    qlo = qlo_of(kt, t)
                    qg = t * QC
                    # S halves always write the full 512 cols (cols below
                    # qlo are masked-region scores that exp/PV never read)
                    # so the exp span is fully initialized by this tile.
                    # On diagonal k-tiles a follow-up identity-matmul
                    # accumulates -1e9 onto the k>q half of the diagonal
                    # block, so exp yields exact zeros there (no mask mul).
                    diag = kt - 4 * t >= 0
                    ps = psS.tile([128, 2 * QC], F32, tag="ps", name="ps")
                    for half, (klo, khi) in enumerate(((0, 64), (64, 128))):
                        nc.tensor.matmul(
                            ps[:, half * QC:(half + 1) * QC],
                            lhsT=kA[klo:khi, kt * KT:(kt + 1) * KT],
                            rhs=qA[klo:khi, qg:qg + QC],
                            start=True, stop=not diag,
                        )
                    if diag:
                        for half in range(2):
                            o = half * QC + qlo
                            nc.tensor.matmul(
                                ps[:, o:o + 128],
                                lhsT=ident[:, :], rhs=maskM[:, :],
                                start=False, stop=True,
                            )
                        cred["pe"] += 160
                    cred["pe"] += 230
                    pT = sbp.tile([128, 2 * QC], BF16, tag="pT", name="pT")
                    nc.scalar.activation(
                        pT[:, qlo:2 * QC], ps[:, qlo:2 * QC], AF.Exp,
                        scale=SCALE,
                    )
                    cred["act"] += (2 * QC - qlo + 352) / 1.2
                    if pend is not None:
                        pkt, ppT = pend
                        pqlo = qlo_of(pkt, t)
                        for h, half in ((0, 0), (1, 1)):
                            nc.tensor.matmul(
                                (py0 if h == 0 else py1)[0:65, pqlo:QC],
                                lhsT=vaug[:, pkt, h * 65:(h + 1) * 65],
                                rhs=ppT[:, half * QC + pqlo:(half + 1) * QC],
                                start=(pkt == 0), stop=(pkt == n_k - 1),
                            )
                        cred["pe"] += 2 * (QC - pqlo) / 2.4
                    pend = (kt, pT)
                    fill()
                pkt, ppT = pend
                pqlo = qlo_of(pkt, t)
                for h, half in ((0, 0), (1, 1)):
                    nc.tensor.matmul(
                        (py0 if h == 0 else py1)[0:65, pqlo:QC],
                        lhsT=vaug[:, pkt, h * 65:(h + 1) * 65],
                        rhs=ppT[:, half * QC + pqlo:(half + 1) * QC],
                        start=(pkt == 0), stop=(pkt == n_k - 1),
                    )
                cred["pe"] += 2 * (QC - pqlo) / 2.4
                emit_norm(0, t, py0)
                emit_norm(1, t, py1)

            # ---- h2 attention block: kt-pairs share one ps tile ----
            def attn2(t):
                n_k = 4 * (t + 1)
                py2 = psY.tile([128, QC], F32, tag="py", name="py")
                pend = None
                for p in range(n_k // 2):
                    kts = (2 * p, 2 * p + 1)
                    qg = t * QC
                    # kt even runs on PE rows 0:63 (K2/Q2 at base 0), kt
                    # odd on rows 64:127 (base 64) -> concurrent row tiles
                    diag = kts[0] - 4 * t >= 0
                    ps = psS.tile([128, 2 * QC], F32, tag="ps", name="ps")
                    nc.tensor.matmul(
                        ps[:, 0:QC],
                        lhsT=k2s[0:64, kts[0] * KT:(kts[0] + 1) * KT],
                        rhs=mCs[0:64, qg:qg + QC],
                        start=True, stop=not diag,
                    )
                    nc.tensor.matmul(
                        ps[:, QC:2 * QC],
                        lhsT=mCs[64:128, kts[1] * KT:(kts[1] + 1) * KT],
                        rhs=k2s[64:128, qg:qg + QC],
                        start=True, stop=not diag,
                    )
                    if diag:
                        for half, kt in enumerate(kts):
                            o = half * QC + qlo_of(kt, t)
                            nc.tensor.matmul(
                                ps[:, o:o + 128],
                                lhsT=ident[:, :], rhs=maskM[:, :],
                                start=False, stop=True,
                            )
                        cred["pe"] += 160
                    cred["pe"] += 230
                    lo = qlo_of(kts[0], t)
                    pT = sbp.tile([128, 2 * QC], BF16, tag="pT", name="pT")
                    nc.scalar.activation(
                        pT[:, lo:2 * QC], ps[:, lo:2 * QC], AF.Exp,
                        scale=SCALE,
                    )
                    cred["act"] += (2 * QC - lo + 352) / 1.2
                    if pend is not None:
                        for pkt, ppT, phalf in pend:
                            pqlo = qlo_of(pkt, t)
                            nc.tensor.matmul(
                                py2[0:65, pqlo:QC],
                                lhsT=vaug[:, pkt, 2 * 65:3 * 65],
                                rhs=ppT[:, phalf * QC + pqlo:(phalf + 1) * QC],
                                start=(pkt == 0), stop=(pkt == n_k - 1),
                            )
                            cred["pe"] += (QC - pqlo) / 2.4
                    pend = [(kts[0], pT, 0), (kts[1], pT, 1)]
                    fill()
                for pkt, ppT, phalf in pend:
                    pqlo = qlo_of(pkt, t)
                    nc.tensor.matmul(
                        py2[0:65, pqlo:QC],
                        lhsT=vaug[:, pkt, 2 * 65:3 * 65],
                        rhs=ppT[:, phalf * QC + pqlo:(phalf + 1) * QC],
                        start=(pkt == 0), stop=(pkt == n_k - 1),
                    )
                    cred["pe"] += (QC - pqlo) / 2.4
                emit_norm(2, t, py2)

            # ---- schedule ----
            # pre-need one block ahead so block boundaries never leave the
            # PE idle long enough for HAM to re-throttle the clock.
            need([("qk", 0, 0), ("qk", 1, 0), ("v", 0), ("v", 1)])
            for t in range(NQC):
                need([("qk", 0, t), ("qk", 1, t)]
                     + [("v", kt) for kt in range(4 * t, 4 * t + 4)]
                     + [("qk", 2, t), ("k2s", t)])
                if t == 0:
                    dma_late_loads()
                attn01(t)
                if t + 1 < NQC:
                    need([("qk", 0, t + 1), ("qk", 1, t + 1)]
                         + [("v", kt) for kt in range(4 * t + 4, 4 * t + 8)])
                attn2(t)
                for ct in range(NCH):
                    filler_q.append(("proj", ct, t))
            # flush whatever the filler scheduler didn't consume; the tail
            # projs alternate their PSUM->SBUF copy onto ACT (idle by now)
            rest = [g for g in filler_q if g not in emitted]
            for g in rest:
                if g[0] != "proj":
                    emit_group(g)
            projs = [g for g in rest if g[0] == "proj"]
            for i, g in enumerate(projs):
                emitted.add(g)
                emit_proj(g[1], g[2], on_act=(i % 2 == 1))

    nc.finalize()
    return nc


def kernel(x, W_attn, b_attn, W_proj, b_proj):
    global LAST_RESULTS
    B = x.shape[0]
    x = np.asarray(x, np.float32)
    W_attn = np.asarray(W_attn, np.float32)
    b_attn = np.asarray(b_attn, np.float32)
    W_proj = np.asarray(W_proj, np.float32)
    b_proj = np.asarray(b_proj, np.float32)

    vbias = bool(np.any(b_attn[2 * C:]))
    key = ("nc", vbias)
    if key not in _CACHE:
        _CACHE[key] = build(vbias=vbias)
    nc = _CACHE[key]

    in_maps = []
    for c in range(8):
        b, g = divmod(c, 4)
        heads = [3 * g + i for i in range(HPC)]
        h0, h1, h2 = heads
        Q = lambda h: W_attn[:, 64 * h:64 * h + 64]
        K = lambda h: W_attn[:, C + 64 * h:C + 64 * h + 64]
        V = lambda h: W_attn[:, 2 * C + 64 * h:2 * C + 64 * h + 64]
        bQ = lambda h: b_attn[64 * h:64 * h + 64]
        bK = lambda h: b_attn[C + 64 * h:C + 64 * h + 64]
        bV = lambda h: b_attn[2 * C + 64 * h:2 * C + 64 * h + 64]
        # m-tiles: [Q0|Q1], [K0|K1], [Q2|K2]
        wqk = np.ascontiguousarray(np.concatenate(
            [Q(h0), Q(h1), K(h0), K(h1), Q(h2), K(h2)], 1)
        ).astype(ml_dtypes.bfloat16)
        wv = np.ascontiguousarray(np.concatenate(
            [V(h0), V(h1), V(h2)], 1)).astype(ml_dtypes.bfloat16)
        bcols = [bQ(h0), bQ(h1), bK(h0), bK(h1), bQ(h2), bK(h2)]
        bvec = np.concatenate(bcols)                      # [384] = 3 x 128
        battn = np.ascontiguousarray(bvec.reshape(3, 128).T)  # [128, 3]
        bvv = np.concatenate([bV(h0), bV(h1), bV(h2)])[None, :]
        wp0 = np.concatenate(
            [W_proj[64 * h:64 * h + 64, :] for h in (h0, h1)], 0)
        wp1 = W_proj[64 * h2:64 * h2 + 64, :]
        xt = np.ascontiguousarray(x[b].T)
        in_maps.append({
            "xTb": xt.astype(ml_dtypes.bfloat16),
            "wqk": wqk,
            "wv": wv,
            "battn": battn,
            "bv": bvv.astype(ml_dtypes.bfloat16),
            "wp0": np.ascontiguousarray(wp0).astype(ml_dtypes.bfloat16),
            "wp1": np.ascontiguousarray(wp1).astype(ml_dtypes.bfloat16),
        })

    res = run_bass_kernel_spmd(nc, in_maps, core_ids=list(range(8)))
    LAST_RESULTS = res

    out = np.zeros((B, T, C), np.float32)
    for c in range(8):
        b = c // 4
        out[b] += res.results[c]["yT"].T
    out += b_proj
    return out


# revision 32
# speedup vs baseline: 1.4061x; 1.0399x over previous
"""Causal self-attention (B=2, T=2048, C=768, H=12) on 8 TRN2 NeuronCores.

Sharding: core c -> batch b = c//4, head-group g = c%4 (heads 3g..3g+2).
Each core computes QKV for its 3 heads, causal attention, and a partial
c_proj (its heads' rows of W_proj). Host sums the 4 partials per batch.

Layout: feature-on-partitions for Q/K (S^T tiles [k, q]), but V is
computed directly in [t, d] layout (lhsT = x^T chunks), which feeds the
PV matmul with no PE transposes. Softmax over k uses an appended
ones-column on V so the PV accumulation yields [y^T; denom] in one
group. No max-subtraction: scores ~N(0,1), exp is fp32-safe.

qk m-tiles (host packs): m0=[Q0|Q1] m1=[K0|K1] m2=[Q2|K2], 128 rows
each so Q_h/K_h of heads 0/1 sit at partition bases 0/64. S matmuls for
h0 (rows 0:64) and h1 (rows 64:128) are emitted back-to-back and run
CONCURRENTLY in different PE row-groups (tile_position auto-derived),
halving S wall time. h2's K2 is copied to a base-0 scratch by GpSimd.

Engine budget per core (est): ACT ~69us (exp, the bottleneck), PE ~71us,
DVE ~50us, GpSimd ~35us. Emission software-pipelines one kt-step ahead
(S(kt+1) before PV(kt)) and a credit-based filler scheduler injects
QKV / V / proj matmul groups between attention steps so PE stays dense
while ACT runs exp. Reciprocal uses reciprocal_approx_fast (5x faster
than DVE reciprocal) on the [1,512] denom row, broadcast by GpSimd.
"""

import os

import numpy as np
import ml_dtypes

import concourse.bass as bass
import concourse.mybir as mybir
import concourse.tile as tile
from concourse import bacc
from concourse.bass_utils import run_bass_kernel_spmd
from concourse.masks import make_identity, make_upper_triangular

F32 = mybir.dt.float32
F32R = mybir.dt.float32r
BF16 = mybir.dt.bfloat16
AF = mybir.ActivationFunctionType

T = 2048           # sequence length
C = 768            # embed dim
HPC = 3            # heads per core
D = 64             # head dim
QC = 512           # q-chunk (psum bank width in fp32)
KT = 128           # k-tile
NKT = T // KT      # 16
NQC = T // QC      # 4
NCH = C // 128     # 6 contraction chunks
SCALE = 1.0 / 8.0  # 1/sqrt(64)

_CACHE = {}
LAST_RESULTS = None
_TCNT = [0]
# debug bisect switches: 'r' safe reciprocal, 'm' masks on vector,
# 'b' skip V-bias matmul
KSAFE = os.environ.get("KSAFE", "")


def mk_persist(pool, shape, dtype, name=None):
    if name is None:
        _TCNT[0] += 1
        name = f"pt{_TCNT[0]}"
    return pool.tile(shape, dtype, name=name, tag=name)


def build(vbias=True):
    nc = bacc.Bacc("TRN2", target_bir_lowering=False)

    xTb = nc.dram_tensor("xTb", [C, T], BF16, kind="ExternalInput")
    wqk = nc.dram_tensor("wqk", [C, 384], BF16, kind="ExternalInput")
    wv = nc.dram_tensor("wv", [C, 192], BF16, kind="ExternalInput")
    battn = nc.dram_tensor("battn", [128, 3], F32, kind="ExternalInput")
    bv = nc.dram_tensor("bv", [1, 192], BF16, kind="ExternalInput")
    wp0 = nc.dram_tensor("wp0", [128, C], BF16, kind="ExternalInput")
    wp1 = nc.dram_tensor("wp1", [64, C], BF16, kind="ExternalInput")
    yT = nc.dram_tensor("yT", [C, T], F32, kind="ExternalOutput")

    with tile.TileContext(nc) as tc, \
            tc.tile_pool(name="persist", bufs=1) as pp:
        # ---- persistent SBUF tensors ----
        xsbb = mk_persist(pp, [128, NCH, T], BF16)     # x^T bf16
        wqk_sb = mk_persist(pp, [128, NCH, 384], BF16)
        wv_sb = mk_persist(pp, [128, NCH, 192], BF16)
        battn_sb = mk_persist(pp, [128, 3], F32)
        bv_sb = mk_persist(pp, [1, 192], BF16)
        ones_r = mk_persist(pp, [1, 128], BF16)        # bias-matmul lhsT
        # Q/K live in bf16: fp32r matmuls stream at half the column rate
        # (fp32_mode=HIGH) and suppress FWL on neighboring ldweights.
        qA = mk_persist(pp, [128, T], BF16)    # [Q0|Q1]
        kA = mk_persist(pp, [128, T], BF16)    # [K0|K1]
        mCs = mk_persist(pp, [128, T], BF16)   # [Q2|K2]
        k2s = mk_persist(pp, [128, T], BF16)   # [K2|Q2] (swapped via DMA)
        vaug = mk_persist(pp, [128, NKT, 3 * 65], BF16)  # V[t,d]+ones per kt
        yA = mk_persist(pp, [128, T], BF16)    # y^T heads 0,1
        yB = mk_persist(pp, [64, T], BF16)     # y^T head 2
        wp0_sb = mk_persist(pp, [128, C], BF16)
        wp1_sb = mk_persist(pp, [64, C], BF16)
        trimask_s = mk_persist(pp, [128, 128], F32)   # [k,q]=1 iff k<=q
        ident_s = mk_persist(pp, [128, 128], F32)
        ident = mk_persist(pp, [128, 128], BF16)   # bf16 -> FWL ldweights
        maskM_s = mk_persist(pp, [128, 128], F32)  # -1e9 where k>q else 0
        maskM = mk_persist(pp, [128, 128], BF16)
        dummy = mk_persist(pp, [1, 8], F32)

        make_upper_triangular(nc, trimask_s[:, :], val=1.0, diag=True)
        make_identity(nc, ident_s[:, :])
        nc.vector.tensor_copy(ident[:, :], ident_s[:, :])
        # maskM = (trimask - 1) * 1e9 : 0 on k<=q, -1e9 on k>q
        nc.vector.tensor_scalar(
            maskM_s[:, :], trimask_s[:, :], -1.0, 1e9,
            op0=mybir.AluOpType.add, op1=mybir.AluOpType.mult,
        )
        nc.vector.tensor_copy(maskM[:, :], maskM_s[:, :])
        nc.vector.memset(ones_r[:, :], 1.0)
        # preload the exp table set while DMAs run
        nc.vector.memset(dummy[:, :], 0.0)
        nc.scalar.activation(dummy[:, :], dummy[:, :], AF.Exp)
        for i in range(3):  # ones columns of vaug
            nc.vector.memset(vaug[:, :, 65 * i + 64:65 * i + 65], 1.0)

        # ---- DMA loads, earliest-needed first, coalesced into few
        # dispatches (each dma_start costs ~600ns of queue dispatch) ----
        nc.sync.dma_start(
            wqk_sb[:, :, 0:256],
            wqk[:, 0:256].rearrange("(c p) m -> p c m", p=128))
        nc.sync.dma_start(
            wv_sb[:, :, :], wv[:, :].rearrange("(c p) m -> p c m", p=128))
        nc.sync.dma_start(
            xsbb[:, :, 0:QC],
            xTb[:, 0:QC].rearrange("(c p) t -> p c t", p=128))
        nc.sync.dma_start(battn_sb[:, :], battn[:, :])
        nc.sync.dma_start(bv_sb[:, :], bv[:, :])
        nc.sync.dma_start(
            wqk_sb[:, :, 256:384],
            wqk[:, 256:384].rearrange("(c p) m -> p c m", p=128))
        nc.sync.dma_start(
            xsbb[:, :, QC:2 * QC],
            xTb[:, QC:2 * QC].rearrange("(c p) t -> p c t", p=128))

        def dma_late_loads():
            # issued from inside the schedule so mid-stream DMAs (k2s,
            # output stores) aren't stuck behind these dispatches
            nc.sync.dma_start(
                xsbb[:, :, 2 * QC:T],
                xTb[:, 2 * QC:T].rearrange("(c p) t -> p c t", p=128))
            nc.sync.dma_start(wp0_sb[:, :], wp0[:, :])
            nc.sync.dma_start(wp1_sb[:, :], wp1[:, :])

        qk_dest = [qA, kA, mCs]

        with (
            tc.tile_pool(name="psS", bufs=2, space="PSUM") as psS,
            tc.tile_pool(name="psY", bufs=2, space="PSUM") as psY,
            tc.tile_pool(name="psM", bufs=2, space="PSUM") as psM,
            tc.tile_pool(name="sb", bufs=8) as sbp,
        ):
            # --- emit helpers; each returns nothing, updates credit ---
            cred = {"pe": 0.0, "act": 0.0}

            def emit_qk(m, t):
                ps = psM.tile([128, QC], F32, tag="pm", name="pm")
                for cc in range(NCH):
                    nc.tensor.matmul(
                        ps[:, :],
                        lhsT=wqk_sb[:, cc, m * 128:(m + 1) * 128],
                        rhs=xsbb[:, cc, t * QC:(t + 1) * QC],
                        start=(cc == 0), stop=(cc == NCH - 1),
                    )
                nc.vector.tensor_scalar_add(
                    qk_dest[m][:, t * QC:(t + 1) * QC], ps[:, :],
                    battn_sb[:, m:m + 1],
                )
                cred["pe"] += 1350

            def emit_k2s(t):
                # partition-swapped copy of mC ([Q2|K2] -> [K2|Q2]): only
                # DMA can move data across partitions cheaply. The swapped
                # tile lets h2's S matmuls run row-paired like h0/h1.
                nc.sync.dma_start(
                    k2s[0:64, t * QC:(t + 1) * QC],
                    mCs[64:128, t * QC:(t + 1) * QC],
                )
                nc.sync.dma_start(
                    k2s[64:128, t * QC:(t + 1) * QC],
                    mCs[0:64, t * QC:(t + 1) * QC],
                )

            def emit_v(kt):
                ps = psM.tile([128, QC], F32, tag="pm", name="pm")
                skip_bias = not vbias or "b" in KSAFE
                for cc in range(NCH):
                    nc.tensor.matmul(
                        ps[:, 0:192],
                        lhsT=xsbb[:, cc, kt * KT:(kt + 1) * KT],
                        rhs=wv_sb[:, cc, :],
                        start=(cc == 0), stop=(skip_bias and cc == NCH - 1),
                    )
                if not skip_bias:
                    nc.tensor.matmul(
                        ps[:, 0:192],
                        lhsT=ones_r[:, :],
                        rhs=bv_sb[:, :],
                        start=False, stop=True,
                    )
                nc.vector.tensor_copy(
                    vaug[:, kt, :].rearrange("p (a b) -> p a b", b=65)[:, :, 0:64],
                    ps[:, 0:192].rearrange("p (a b) -> p a b", b=64),
                )
                cred["pe"] += 640

            def emit_proj(ct, t, on_act=False):
                ps = psM.tile([128, QC], F32, tag="pm", name="pm")
                nc.tensor.matmul(
                    ps[:, :],
                    lhsT=wp0_sb[:, ct * 128:(ct + 1) * 128],
                    rhs=yA[:, t * QC:(t + 1) * QC],
                    start=True, stop=False,
                )
                nc.tensor.matmul(
                    ps[:, :],
                    lhsT=wp1_sb[0:64, ct * 128:(ct + 1) * 128],
                    rhs=yB[0:64, t * QC:(t + 1) * QC],
                    start=False, stop=True,
                )
                osb = sbp.tile([128, QC], F32, tag="osb", name="osb")
                if on_act:
                    nc.scalar.activation(osb[:, :], ps[:, :], AF.Copy)
                else:
                    nc.vector.tensor_copy(osb[:, :], ps[:, :])
                nc.sync.dma_start(
                    yT[ct * 128:(ct + 1) * 128, t * QC:(t + 1) * QC],
                    osb[:, :],
                )
                cred["pe"] += 500

            # ---- filler scheduler ----
            emitted = set()

            def emit_group(g):
                if g in emitted:
                    return
                emitted.add(g)
                kind = g[0]
                if kind == "qk":
                    emit_qk(g[1], g[2])
                elif kind == "v":
                    emit_v(g[1])
                elif kind == "k2s":
                    emit_k2s(g[1])
                elif kind == "proj":
                    emit_proj(g[1], g[2])

            filler_q = []

            def fill():
                while filler_q and cred["pe"] < cred["act"]:
                    emit_group(filler_q.pop(0))

            # filler order: deadline-sorted supply of PE work
            for kt in range(2, 8):
                filler_q.append(("v", kt))
            filler_q.append(("qk", 2, 0))    # mC t0 (needed by C(0))
            filler_q.append(("k2s", 0))
            filler_q += [("qk", 1, 1), ("qk", 0, 1)]
            for kt in range(8, 12):
                filler_q.append(("v", kt))
            filler_q += [("qk", 2, 1), ("k2s", 1)]
            filler_q += [("qk", 1, 2), ("qk", 0, 2)]
            for kt in range(12, 16):
                filler_q.append(("v", kt))
            filler_q += [("qk", 2, 2), ("k2s", 2)]
            filler_q += [("qk", 1, 3), ("qk", 0, 3)]
            filler_q += [("qk", 2, 3), ("k2s", 3)]

            def need(groups):
                for g in groups:
                    if g not in emitted:
                        if g in filler_q:
                            filler_q.remove(g)
                        emit_group(g)

            def qlo_of(kt, t):
                dm = kt - 4 * t
                return 128 * dm if dm >= 0 else 0

            def emit_norm(h, t, py):
                ydest, yrow = (yA, 0) if h == 0 else (yA, 64) if h == 1 else (yB, 0)
                if "r" in KSAFE:
                    den = sbp.tile([1, QC], F32, tag="rec", name="rec")
                    nc.vector.tensor_copy(den[:, :], py[64:65, :])
                    bc = sbp.tile([64, QC], F32, tag="bc", name="bc")
                    nc.gpsimd.partition_broadcast(bc[:, :], den[0:1, :])
                    rec64 = sbp.tile([64, QC], F32, tag="rec64", name="rec64")
                    nc.vector.reciprocal(rec64[:, :], bc[:, :])
                    nc.vector.tensor_mul(
                        ydest[yrow:yrow + 64, t * QC:(t + 1) * QC],
                        py[0:64, :], rec64[:, :],
                    )
                    return
                # reciprocal_approx_fast mis-reads PSUM at partition offset
                # 64 on HW (unit-tested) -> stage the denom row to SBUF
                # partition 0 first (baseline-proven DVE row move).
                den = sbp.tile([1, QC], F32, tag="den", name="den")
                nc.vector.tensor_copy(den[:, :], py[64:65, :])
                rec = sbp.tile([1, QC], F32, tag="rec", name="rec")
                nc.vector.reciprocal_approx_fast(rec[:, :], den[:, :])
                bc = sbp.tile([64, QC], F32, tag="bc", name="bc")
                nc.gpsimd.partition_broadcast(bc[:, :], rec[0:1, :])
                nc.vector.tensor_mul(
                    ydest[yrow:yrow + 64, t * QC:(t + 1) * QC],
                    py[0:64, :], bc[:, :],
                )

            # ---- h01 attention block: heads 0,1 row-paired per kt ----
            def attn01(t):
                n_k = 4 * (t + 1)
                py0 = psY.tile([128, QC], F32, tag="py", name="py")
                py1 = psY.tile([128, QC], F32, tag="py", name="py")
                pend = None  # (kt, pT)
                for kt in range(n_k):
                    qlo = qlo_of(kt, t)
                    qg = t * QC
                    # S halves always write the full 512 cols (cols below
                    # qlo are masked-region scores that exp/PV never read)
                    # so the exp span is fully initialized by this tile.
                    # On diagonal k-tiles a follow-up identity-matmul
                    # accumulates -1e9 onto the k>q half of the diagonal
                    # block, so exp yields exact zeros there (no mask mul).
                    diag = kt - 4 * t >= 0
                    ps = psS.tile([128, 2 * QC], F32, tag="ps", name="ps")
                    for half, (klo, khi) in enumerate(((0, 64), (64, 128))):
                        nc.tensor.matmul(
                            ps[:, half * QC:(half + 1) * QC],
                            lhsT=kA[klo:khi, kt * KT:(kt + 1) * KT],
                            rhs=qA[klo:khi, qg:qg + QC],
                            start=True, stop=not diag,
                        )
                    if diag:
                        for half in range(2):
                            o = half * QC + qlo
                            nc.tensor.matmul(
                                ps[:, o:o + 128],
                                lhsT=ident[:, :], rhs=maskM[:, :],
                                start=False, stop=True,
                            )
                        cred["pe"] += 160
                    cred["pe"] += 230
                    pT = sbp.tile([128, 2 * QC], BF16, tag="pT", name="pT")
                    nc.scalar.activation(
                        pT[:, qlo:2 * QC], ps[:, qlo:2 * QC], AF.Exp,
                        scale=SCALE,
                    )
                    cred["act"] += (2 * QC - qlo + 352) / 1.2
                    if pend is not None:
                        pkt, ppT = pend
                        pqlo = qlo_of(pkt, t)
                        for h, half in ((0, 0), (1, 1)):
                            nc.tensor.matmul(
                                (py0 if h == 0 else py1)[0:65, pqlo:QC],
                                lhsT=vaug[:, pkt, h * 65:(h + 1) * 65],
                                rhs=ppT[:, half * QC + pqlo:(half + 1) * QC],
                                start=(pkt == 0), stop=(pkt == n_k - 1),
                            )
                        cred["pe"] += 2 * (QC - pqlo) / 2.4
                    pend = (kt, pT)
                    fill()
                pkt, ppT = pend
                pqlo = qlo_of(pkt, t)
                for h, half in ((0, 0), (1, 1)):
                    nc.tensor.matmul(
                        (py0 if h == 0 else py1)[0:65, pqlo:QC],
                        lhsT=vaug[:, pkt, h * 65:(h + 1) * 65],
                        rhs=ppT[:, half * QC + pqlo:(half + 1) * QC],
                        start=(pkt == 0), stop=(pkt == n_k - 1),
                    )
                cred["pe"] += 2 * (QC - pqlo) / 2.4
                emit_norm(0, t, py0)
                emit_norm(1, t, py1)

            # ---- h2 attention block: kt-pairs share one ps tile ----
            def attn2(t):
                n_k = 4 * (t + 1)
                py2 = psY.tile([128, QC], F32, tag="py", name="py")
                pend = None
                for p in range(n_k // 2):
                    kts = (2 * p, 2 * p + 1)
                    qg = t * QC
                    # kt even runs on PE rows 0:63 (K2/Q2 at base 0), kt
                    # odd on rows 64:127 (base 64) -> concurrent row tiles
                    diag = kts[0] - 4 * t >= 0
                    ps = psS.tile([128, 2 * QC], F32, tag="ps", name="ps")
                    nc.tensor.matmul(
                        ps[:, 0:QC],
                        lhsT=k2s[0:64, kts[0] * KT:(kts[0] + 1) * KT],
                        rhs=mCs[0:64, qg:qg + QC],
                        start=True, stop=not diag,
                    )
                    nc.tensor.matmul(
                        ps[:, QC:2 * QC],
                        lhsT=mCs[64:128, kts[1] * KT:(kts[1] + 1) * KT],
                        rhs=k2s[64:128, qg:qg + QC],
                        start=True, stop=not diag,
                    )
                    if diag:
                        for half, kt in enumerate(kts):
                            o = half * QC + qlo_of(kt, t)
                            nc.tensor.matmul(
                                ps[:, o:o + 128],
                                lhsT=ident[:, :], rhs=maskM[:, :],
                                start=False, stop=True,
                            )
                        cred["pe"] += 160
                    cred["pe"] += 230
                    lo = qlo_of(kts[0], t)
                    pT = sbp.tile([128, 2 * QC], BF16, tag="pT", name="pT")
                    nc.scalar.activation(
                        pT[:, lo:2 * QC], ps[:, lo:2 * QC], AF.Exp,
                        scale=SCALE,
                    )
                    cred["act"] += (2 * QC - lo + 352) / 1.2
                    if pend is not None:
                        for pkt, ppT, phalf in pend:
                            pqlo = qlo_of(pkt, t)
                            nc.tensor.matmul(
                                py2[0:65, pqlo:QC],
                                lhsT=vaug[:, pkt, 2 * 65:3 * 65],
                                rhs=ppT[:, phalf * QC + pqlo:(phalf + 1) * QC],
                                start=(pkt == 0), stop=(pkt == n_k - 1),
                            )
                            cred["pe"] += (QC - pqlo) / 2.4
                    pend = [(kts[0], pT, 0), (kts[1], pT, 1)]
                    fill()
                for pkt, ppT, phalf in pend:
                    pqlo = qlo_of(pkt, t)
                    nc.tensor.matmul(
                        py2[0:65, pqlo:QC],
                        lhsT=vaug[:, pkt, 2 * 65:3 * 65],
                        rhs=ppT[:, phalf * QC + pqlo:(phalf + 1) * QC],
                        start=(pkt == 0), stop=(pkt == n_k - 1),
                    )
                    cred["pe"] += (QC - pqlo) / 2.4
                emit_norm(2, t, py2)

            # ---- schedule ----
            # pre-need one block ahead so block boundaries never leave the
            # PE idle long enough for HAM to re-throttle the clock.
            need([("qk", 0, 0), ("qk", 1, 0), ("v", 0), ("v", 1)])
            for t in range(NQC):
                need([("qk", 0, t), ("qk", 1, t)]
                     + [("v", kt) for kt in range(4 * t, 4 * t + 4)]
                     + [("qk", 2, t), ("k2s", t)])
                if t == 0:
                    dma_late_loads()
                attn01(t)
                if t + 1 < NQC:
                    need([("qk", 0, t + 1), ("qk", 1, t + 1)]
                         + [("v", kt) for kt in range(4 * t + 4, 4 * t + 8)])
                attn2(t)
                for ct in range(NCH):
                    filler_q.append(("proj", ct, t))
            # flush whatever the filler scheduler didn't consume; the tail
            # projs alternate their PSUM->SBUF copy onto ACT (idle by now)
            rest = [g for g in filler_q if g not in emitted]
            for g in rest:
                if g[0] != "proj":
                    emit_group(g)
            projs = [g for g in rest if g[0] == "proj"]
            for i, g in enumerate(projs):
                emitted.add(g)
                emit_proj(g[1], g[2], on_act=(i >= len(projs) - 3))

    nc.finalize()
    return nc


def kernel(x, W_attn, b_attn, W_proj, b_proj):
    global LAST_RESULTS
    B = x.shape[0]
    x = np.asarray(x, np.float32)
    W_attn = np.asarray(W_attn, np.float32)
    b_attn = np.asarray(b_attn, np.float32)
    W_proj = np.asarray(W_proj, np.float32)
    b_proj = np.asarray(b_proj, np.float32)

    vbias = bool(np.any(b_attn[2 * C:]))
    key = ("nc", vbias)
    if key not in _CACHE:
        _CACHE[key] = build(vbias=vbias)
    nc = _CACHE[key]

    in_maps = []
    for c in range(8):
        b, g = divmod(c, 4)
        heads = [3 * g + i for i in range(HPC)]
        h0, h1, h2 = heads
        Q = lambda h: W_attn[:, 64 * h:64 * h + 64]
        K = lambda h: W_attn[:, C + 64 * h:C + 64 * h + 64]
        V = lambda h: W_attn[:, 2 * C + 64 * h:2 * C + 64 * h + 64]
        bQ = lambda h: b_attn[64 * h:64 * h + 64]
        bK = lambda h: b_attn[C + 64 * h:C + 64 * h + 64]
        bV = lambda h: b_attn[2 * C + 64 * h:2 * C + 64 * h + 64]
        # m-tiles: [Q0|Q1], [K0|K1], [Q2|K2]
        wqk = np.ascontiguousarray(np.concatenate(
            [Q(h0), Q(h1), K(h0), K(h1), Q(h2), K(h2)], 1)
        ).astype(ml_dtypes.bfloat16)
        wv = np.ascontiguousarray(np.concatenate(
            [V(h0), V(h1), V(h2)], 1)).astype(ml_dtypes.bfloat16)
        bcols = [bQ(h0), bQ(h1), bK(h0), bK(h1), bQ(h2), bK(h2)]
        bvec = np.concatenate(bcols)                      # [384] = 3 x 128
        battn = np.ascontiguousarray(bvec.reshape(3, 128).T)  # [128, 3]
        bvv = np.concatenate([bV(h0), bV(h1), bV(h2)])[None, :]
        wp0 = np.concatenate(
            [W_proj[64 * h:64 * h + 64, :] for h in (h0, h1)], 0)
        wp1 = W_proj[64 * h2:64 * h2 + 64, :]
        xt = np.ascontiguousarray(x[b].T)
        in_maps.append({
            "xTb": xt.astype(ml_dtypes.bfloat16),
            "wqk": wqk,
            "wv": wv,
            "battn": battn,
            "bv": bvv.astype(ml_dtypes.bfloat16),
            "wp0": np.ascontiguousarray(wp0).astype(ml_dtypes.bfloat16),
            "wp1": np.ascontiguousarray(wp1).astype(ml_dtypes.bfloat16),
        })

    res = run_bass_kernel_spmd(nc, in_maps, core_ids=list(range(8)))
    LAST_RESULTS = res

    out = np.zeros((B, T, C), np.float32)
    for c in range(8):
        b = c // 4
        out[b] += res.results[c]["yT"].T
    out += b_proj
    return out
